# revision 46
# baseline (speedup 1.0000x reference)
"""CosFace (LMCL) loss + center loss, sharded over 8 Trainium2 NeuronCores.

Strategy (classification parallel over the class dim):
  - weight [50000,128] is zero-padded to [50176,128], split into 8 shards of
    6272 rows, and each shard is uploaded PRE-TRANSPOSED as bf16 wT [128,6272]
    (raw, unnormalized values - normalization happens on device).
  - On-device weight normalization: SQ = wT*wT (DVE, bf16 2x), per-class
    norms^2 via tiny PE matmuls (stationary = SQ tile, moving = ones column),
    1/norm via ACT Ln/Exp on a [128,49] tile, PE-transpose of the scale tile,
    SWDGE broadcast-DMA to replicate scales across partitions, then a fused
    multiply produces normalized wts (bf16).
  - Features are normalized in natural layout and PE-transposed into
    fT bf16 [128,1024] (built per 128-sample chunk for early pipelining).
  - Main loop: per chunk the 6272 local classes stream through PSUM in four
    fills (2048,2048,2048,128).  Fills 0-1 are consumed by ScalarE:
    exp(s*cos - 30) fused with accumulation (accum_out).  Fills 2-3 are
    consumed by VectorE using a Schraudolph bit-trick exp: one
    scalar_tensor_tensor computes int16(cos*K1 + K2) (the bf16 bit pattern of
    ~exp(s*cos - 30)), then one tensor_tensor_reduce on the bitcast values
    accumulates them at 2x rate.  This splits the 6.4M-element exp between
    the two engines.
  - The center-loss/target-cosine path (t, q) is sharded: each core handles
    only its own 128 samples from host-gathered fmy/wl tiles (Pool engine).
  - Host combines in float64: sums partial exp-sums across cores, subtracts
    the exact padding contribution, applies the CosFace margin correction,
    and assembles loss = mean(lse - s*(t-m)) + lambda*0.5*sum(q).
"""

import math

import ml_dtypes
import numpy as np

import concourse.bass as bass
import concourse.mybir as mybir
import concourse.tile as tile
from concourse.bass_utils import run_bass_kernel_spmd
from concourse.masks import make_identity

# ---------------------------------------------------------------------------
# Workaround for this container's walrus build: instructions carrying more
# than one semaphore wait fail codegen.  Move all but one wait onto
# standalone single-wait EventSemaphore instructions inserted immediately
# before, on the same engine.
# ---------------------------------------------------------------------------


def _split_multi_waits(nc):
    for fn in nc.m.functions:
        for bb in fn.blocks:
            insts = bb.instructions
            out = []
            changed = False
            for inst in insts:
                si = inst.sync_info
                if si is not None and len(si.on_wait) > 1:
                    waits = list(si.on_wait)
                    for w in waits[:-1]:
                        ev = mybir.InstEventSemaphore(
                            name=nc.get_next_instruction_name(), ins=[], outs=[]
                        )
                        ev.engine = inst.engine
                        ev.sync_info = mybir.SyncInfo(on_wait=[w], on_update=[])
                        nc.register_instruction(ev, overwrite=True)
                        out.append(ev)
                    si.on_wait[:] = waits[-1:]
                    changed = True
                out.append(inst)
            if changed:
                bb.instructions = out

# ---------------------------------------------------------------------------

F32 = mybir.dt.float32
BF16 = mybir.dt.bfloat16
I16 = mybir.dt.int16
AF = mybir.ActivationFunctionType
AX = mybir.AxisListType
OP = mybir.AluOpType

N_CORES = 8
N = 1024
C = 50000
D = 128
P = 128
NCH = N // P  # 8 sample chunks
CT = 49  # class tiles per core
CLOC = CT * P  # 6272 local classes
CPAD = N_CORES * CLOC  # 50176
NPAD = CPAD - C  # 176 zero rows (all on the last core)

S_SCALE = 30.0
M_MARGIN = 0.35
LAMBDA = 0.01
EXP_BIAS = -30.0  # exp(s*cos + EXP_BIAS); s*cos <= 30 so sums stay in fp32
EPS2 = 1e-16  # matches torch CosineSimilarity eps=1e-8 on squared norms

# Schraudolph bf16 exp: bitpattern(e^z) ~ int16(z*(2^7/ln2) + 2^7*(127-c)).
SCH_A = 128.0 / math.log(2.0)  # 184.6650
SCH_C = 0.0430
K1S = SCH_A * S_SCALE  # applied to cos directly
K2P = 128.0 * (127.0 - SCH_C) + SCH_A * EXP_BIAS  # folds the -30 bias


def _schraudolph_exp_np(cos):
    """Host replica of the device Schraudolph path (for the pad correction)."""
    i = np.round(np.float32(cos) * np.float32(K1S) + np.float32(K2P))
    return np.asarray(i.astype(np.int16).view(ml_dtypes.bfloat16), np.float64)


# class blocks: columns [o0, o1) and the engine that consumes each block.
# Pass 1 consumes blocks 0 (ACT) + 1 (DVE); pass 2 blocks 2 (ACT) + 3 (DVE).
# Sizes balance ACT (1 col/cycle @1.2GHz + accum fixed) against the DVE
# Schraudolph path (1x affine from PSUM + 4x accumulate).
BLOCKS = [(0, 2048), (2048, 3584), (3584, 5248), (5248, 6272)]
ACT_BLOCKS = (0, 2)
DVE_BLOCKS = (1, 3)
PASS_BLOCKS = [(0, 1), (2, 3)]
# norm-prep group g covers the blocks of pass g (prep of group 1 is emitted
# after pass 1's fills so it overlaps execution).
NORM_GROUPS = [(0, 1), (2, 3)]
MMB = 512  # matmul moving-block width (bass cap)


def _build_program(loop_iters=None):
    nc = bass.Bass(
        "TRN2", target_bir_lowering=False, debug=False, num_devices=N_CORES
    )
    wt_d = nc.dram_tensor("wt", [D, CLOC], BF16, kind="ExternalInput").ap()
    ft_d = nc.dram_tensor("ft", [D, N], BF16, kind="ExternalInput").ap()
    f_d = nc.dram_tensor("f", [N, D], F32, kind="ExternalInput").ap()
    fmy_d = nc.dram_tensor("fmy", [P, D], F32, kind="ExternalInput").ap()
    wl_d = nc.dram_tensor("wl", [P, D], F32, kind="ExternalInput").ap()
    rt_d = nc.dram_tensor("rt_scratch", [4, 16, P], BF16, kind="Internal").ap()
    rf_d = nc.dram_tensor("rf_scratch", [NCH, P], BF16, kind="Internal").ap()
    NOUT = 4 * NCH + 2  # 2 ACT accs + 2 DVE accs per chunk, then t, q
    o_d = nc.dram_tensor("o", [P, NOUT], F32, kind="ExternalOutput").ap()

    with tile.TileContext(nc) as tc:
        from contextlib import ExitStack, nullcontext

        with ExitStack() as ctx:
            cpool = ctx.enter_context(tc.tile_pool(name="const", bufs=1))
            big = ctx.enter_context(tc.tile_pool(name="big", bufs=1))
            st = ctx.enter_context(tc.tile_pool(name="stats", bufs=1))
            scr = ctx.enter_context(tc.tile_pool(name="scr", bufs=2))
            psum = ctx.enter_context(
                tc.tile_pool(name="psum", bufs=2, space="PSUM")
            )

            # ---- constants (outside the timing loop) ---------------------
            ident = cpool.tile([P, P], F32, tag="ident")
            make_identity(nc, ident[:])
            ebias = cpool.tile([P, 1], F32, tag="ebias")
            nc.gpsimd.memset(ebias[:], EXP_BIAS)
            ones_col = cpool.tile([P, 1], BF16, tag="ones_col")
            nc.gpsimd.memset(ones_col[:], 1.0)
            pass
            dummy = cpool.tile([P, 1], F32, tag="dummy")
            ABL = "b"

            # ---- big tiles ------------------------------------------------
            wt = big.tile([P, CLOC], BF16, tag="wt")  # raw transposed weights
            sq = big.tile([P, CLOC], BF16, tag="sq")  # squares
            wts = big.tile([P, CLOC], BF16, tag="wts")  # normalized weights
            bsc = big.tile([P, CLOC], BF16, tag="bsc")  # broadcast scales
            f_nat = big.tile([P, N], F32, tag="f_nat")
            fsq = big.tile([P, N], F32, tag="fsq")
            fT = big.tile([P, N], BF16, tag="fT")  # raw transposed features
            fTs = big.tile([P, N], BF16, tag="fTs")  # normalized
            bscf = big.tile([P, N], BF16, tag="bscf")

            # ---- stats / small tiles -------------------------------------
            fns = st.tile([P, NCH], F32, tag="fns")
            fln = st.tile([P, NCH], F32, tag="fln")
            rf = st.tile([P, NCH], F32, tag="rf")
            ns = st.tile([P, 64], F32, tag="ns")  # norms^2, grouped cols
            rw = st.tile([P, 64], F32, tag="rw")  # 1/norm, grouped cols
            rT = st.tile([P, 5 * P], BF16, tag="rT")  # transposed scales
            fmy = st.tile([P, D], F32, tag="fmy")
            wl = st.tile([P, D], F32, tag="wl")
            pq = st.tile([P, 5 * D], F32, tag="pq")  # wl-path scratch
            nrm2 = st.tile([P, 2], F32, tag="nrm2")
            nrml = st.tile([P, 2], F32, tag="nrml")
            rfw = st.tile([P, 2], F32, tag="rfw")
            dots = st.tile([P, 1], F32, tag="dots")
            tt = st.tile([P, 1], F32, tag="tt")
            outt = st.tile([P, NOUT], F32, tag="outt")

            # DVE-path scratch (double-buffered via pool)
            eint = [
                scr.tile([P, 2048], I16, tag="eint", name=f"eint{i}")
                for i in range(2)
            ]
            edum = [
                scr.tile([P, 2048], BF16, tag="edum", name=f"edum{i}")
                for i in range(2)
            ]

            if ABL == "a":
                nc.gpsimd.memset(rT[:], 0.088)
            if ABL == "b":
                nc.gpsimd.memset(bsc[:], 0.088)
            f3d_dram = f_d.rearrange("(c p) d -> p c d", p=P)
            f_nat3 = f_nat[:].rearrange("p (c d) -> p c d", d=D)
            sq3f = lambda ap: ap.rearrange("p (c d) -> p c d", d=D)

            # ---- ACT table warm ------------------------------------------
            nc.scalar.activation(dummy[:], ebias[:], AF.Exp)

            loop_cm = (
                tc.For_i(
                    0,
                    loop_iters,
                    1,
                    hint_engines=(
                        mybir.EngineType.PE,
                        mybir.EngineType.Activation,
                        mybir.EngineType.DVE,
                    ),
                )
                if loop_iters is not None
                else nullcontext()
            )
            _ = loop_cm.__enter__()

            # ---- input DMAs (two HWDGE rings: sync=SP, scalar=ACT) --------
            # sync ring carries the pass-1-critical loads + the scale
            # bounce/broadcast; scalar ring carries the rest in parallel.
            nc.sync.dma_start(fT[:], ft_d[:])
            for bi in (0, 1):
                o0, o1 = BLOCKS[bi]
                nc.sync.dma_start(wt[:, o0:o1], wt_d[:, o0:o1])
            for bi in (2, 3):
                o0, o1 = BLOCKS[bi]
                nc.scalar.dma_start(wt[:, o0:o1], wt_d[:, o0:o1])
            nc.scalar.dma_start(f_nat3[:], f3d_dram[:])
            nc.scalar.dma_start(fmy[:], fmy_d[:])
            nc.scalar.dma_start(wl[:], wl_d[:])

            # ---- feature stats; fTs = fT * broadcast(1/|f|) ---------------
            nc.vector.tensor_mul(fsq[:], f_nat[:], f_nat[:])
            nc.vector.reduce_sum(fns[:], sq3f(fsq[:]), axis=AX.X)
            nc.vector.tensor_scalar_max(fns[:], fns[:], EPS2)
            nc.scalar.activation(fln[:], fns[:], AF.Ln)
            nc.scalar.activation(rf[:], fln[:], AF.Exp, scale=-0.5)
            prf = psum.tile([P, 2048], F32, tag="ps")
            nc.tensor.transpose(prf[:NCH, :P], rf[:], ident[:])
            nc.vector.tensor_copy(rT[:NCH, 4 * P : 5 * P], prf[:NCH, :P])
            nc.sync.dma_start(rf_d[:, :], rT[:NCH, 4 * P : 5 * P])
            srcf = rf_d[:, :].unsqueeze(0).broadcast_to((P, NCH, P))
            dstf = bscf[:].rearrange("p (t c) -> p t c", c=P)
            nc.sync.dma_start(dstf, srcf)
            nc.vector.tensor_mul(fTs[:], fT[:], bscf[:])

            # ---- weight norms, banded in two groups ----------------------
            def w_norm_group(g):
                blocks = NORM_GROUPS[g]
                base = 32 * g
                for bi in blocks:
                    o0, o1 = BLOCKS[bi]
                    nc.vector.tensor_mul(
                        sq[:, o0:o1], wt[:, o0:o1], wt[:, o0:o1]
                    )
                if ABL != "a":
                    pns = psum.tile([P, 2048], F32, tag="ps")
                cols_of = {}
                col = 0
                for bi in blocks:
                    o0, o1 = BLOCKS[bi]
                    nt = (o1 - o0) // P
                    cols_of[bi] = (col, col + nt)
                    for t in range(nt):
                        if ABL != "a":
                            nc.tensor.matmul(
                                pns[:, col : col + 1],
                                sq[:, o0 + t * P : o0 + (t + 1) * P],
                                ones_col[:],
                                start=True,
                                stop=True,
                            )
                        col += 1
                if ABL != "a":
                    nc.vector.tensor_copy(ns[:, base : base + col], pns[:, :col])
                    nc.vector.tensor_scalar_max(
                        ns[:, base : base + col], ns[:, base : base + col], EPS2
                    )
                    nc.scalar.activation(
                        rw[:, base : base + col], ns[:, base : base + col], AF.Ln
                    )
                    nc.scalar.activation(
                        rw[:, base : base + col],
                        rw[:, base : base + col],
                        AF.Exp,
                        scale=-0.5,
                    )
                    prt = psum.tile([P, 2048], F32, tag="ps")
                for bi in blocks:
                    c0, c1 = cols_of[bi]
                    nt = c1 - c0
                    if ABL != "a":
                        nc.tensor.transpose(
                            prt[:nt, bi * P : bi * P + P],
                            rw[:, base + c0 : base + c1],
                            ident[:],
                        )
                        nc.vector.tensor_copy(
                            rT[:nt, bi * P : bi * P + P],
                            prt[:nt, bi * P : bi * P + P],
                        )
                    if ABL != "b":
                        nc.sync.dma_start(
                            rt_d[bi, :nt, :], rT[:nt, bi * P : bi * P + P]
                        )
                        o0, o1 = BLOCKS[bi]
                        src = (
                            rt_d[bi, :nt, :].unsqueeze(0).broadcast_to((P, nt, P))
                        )
                        dst = bsc[:, o0:o1].rearrange("p (t c) -> p t c", c=P)
                        nc.sync.dma_start(dst, src)

            def w_scale(bi):
                o0, o1 = BLOCKS[bi]
                nc.vector.tensor_mul(
                    wts[:, o0:o1], wt[:, o0:o1], bsc[:, o0:o1]
                )

            w_norm_group(0)
            w_scale(0)
            w_scale(1)
            w_norm_group(1)

            # ---- main loop ------------------------------------------------
            def fill_and_consume(ch, bi, sidx):
                o0, o1 = BLOCKS[bi]
                gw = o1 - o0
                pt = psum.tile([P, 2048], F32, tag="ps")
                lhs = fTs[:, ch * P : (ch + 1) * P]
                for b in range(0, gw, MMB):
                    bw = min(MMB, gw - b)
                    nc.tensor.matmul(
                        pt[:, b : b + bw],
                        lhs,
                        wts[:, o0 + b : o0 + b + bw],
                        start=True,
                        stop=True,
                    )
                acc = outt[:, 4 * ch + bi : 4 * ch + bi + 1]
                if bi in ACT_BLOCKS:
                    nc.scalar.activation(
                        pt[:, :gw],
                        pt[:, :gw],
                        AF.Exp,
                        bias=ebias[:],
                        scale=S_SCALE,
                        accum_out=acc,
                    )
                else:
                    ei = eint[sidx][:, :gw]
                    ed = edum[sidx][:, :gw]
                    nc.vector.tensor_scalar(
                        ei, pt[:, :gw], K1S, K2P, OP.mult, OP.add
                    )
                    ev = ei.bitcast(BF16)
                    nc.vector.tensor_scalar(
                        ed, ev, 1.0, 0.0, OP.mult, OP.add, accum_out=acc
                    )

            # pass 1: blocks 0 (ACT) and 1 (DVE) for all chunks; the pass-2
            # scale multiplies are slipped in early so they finish under it
            for ch in range(NCH):
                fill_and_consume(ch, 0, 0)
                fill_and_consume(ch, 1, ch % 2)
                if ch == 1:
                    w_scale(2)
                elif ch == 3:
                    w_scale(3)

            # ---- wl-path (own 128 samples only) --------------------------
            pq5 = pq[:].rearrange("p (k d) -> p k d", d=D)
            nc.vector.tensor_mul(pq5[:, 0, :], fmy[:], fmy[:])
            nc.vector.tensor_mul(pq5[:, 1, :], wl[:], wl[:])
            nc.vector.reduce_sum(nrm2[:], pq5[:, 0:2, :], axis=AX.X)
            nc.vector.tensor_scalar_max(nrm2[:], nrm2[:], EPS2)
            nc.scalar.activation(nrml[:], nrm2[:], AF.Ln)
            nc.scalar.activation(rfw[:], nrml[:], AF.Exp, scale=-0.5)
            nc.vector.tensor_mul(pq5[:, 2, :], fmy[:], wl[:])
            nc.vector.reduce_sum(dots[:], pq5[:, 2:3, :], axis=AX.X)
            nc.vector.tensor_sub(pq5[:, 3, :], fmy[:], wl[:])
            nc.vector.tensor_mul(pq5[:, 4, :], pq5[:, 3, :], pq5[:, 3, :])
            nc.vector.reduce_sum(
                outt[:, 4 * NCH + 1 : 4 * NCH + 2], pq5[:, 4:5, :], axis=AX.X
            )
            nc.vector.tensor_mul(tt[:], dots[:], rfw[:, 0:1])
            nc.vector.tensor_mul(
                outt[:, 4 * NCH : 4 * NCH + 1], tt[:], rfw[:, 1:2]
            )

            # pass 2: blocks 2 (ACT) and 3 (DVE) for all chunks
            for ch in range(NCH):
                fill_and_consume(ch, 2, 0)
                fill_and_consume(ch, 3, ch % 2)

            nc.sync.dma_start(o_d[:], outt[:])

            loop_cm.__exit__(None, None, None)

    _split_multi_waits(nc)
    return nc


_NC_CACHE = None


def _get_program():
    global _NC_CACHE
    if _NC_CACHE is None:
        _NC_CACHE = _build_program()
    return _NC_CACHE


def _build_program_loop(iters):
    return _build_program(loop_iters=iters)


def _host_prepare(feature, weight, label):
    feature = np.ascontiguousarray(np.asarray(feature, dtype=np.float32))
    weight = np.asarray(weight, dtype=np.float32)
    label = np.asarray(label).astype(np.int64)
    wl = np.ascontiguousarray(weight[label])
    w_pad = np.zeros((CPAD, D), dtype=np.float32)
    w_pad[:C] = weight
    in_maps = []
    ft = np.ascontiguousarray(feature.T.astype(ml_dtypes.bfloat16))
    for k in range(N_CORES):
        shard = w_pad[k * CLOC : (k + 1) * CLOC]
        wt = np.ascontiguousarray(shard.T.astype(ml_dtypes.bfloat16))
        in_maps.append(
            {
                "wt": wt,
                "ft": ft,
                "f": feature,
                "fmy": np.ascontiguousarray(feature[k * P : (k + 1) * P]),
                "wl": np.ascontiguousarray(wl[k * P : (k + 1) * P]),
            }
        )
    return in_maps


def _host_combine(results):
    # device layout: out[p, col]; sample n = ch*128 + p
    outs = [np.asarray(r["o"], dtype=np.float64) for r in results]
    A = sum(o[:, : 4 * NCH].reshape(P, NCH, 4).sum(axis=2) for o in outs)
    A_n = A.T.reshape(N)  # [1024]
    t_n = np.concatenate([o[:, 4 * NCH] for o in outs])  # [1024]
    q_n = np.concatenate([o[:, 4 * NCH + 1] for o in outs])

    # padded classes live in the DVE (Schraudolph) blocks of the last core:
    # each contributes the exact bit-trick value of exp(s*0 - 30).
    pad_term = _schraudolph_exp_np(np.zeros(1))[0]
    S_raw = A_n - NPAD * pad_term
    S_fix = (
        S_raw
        - np.exp(S_SCALE * t_n + EXP_BIAS)
        + np.exp(S_SCALE * (t_n - M_MARGIN) + EXP_BIAS)
    )
    lse = np.log(S_fix) - EXP_BIAS
    target_logit = S_SCALE * (t_n - M_MARGIN)
    loss_lmc = np.mean(lse - target_logit)
    loss_c = 0.5 * np.sum(q_n)
    return np.float32(loss_lmc + LAMBDA * loss_c)


def kernel(feature, weight, label):
    nc = _get_program()
    in_maps = _host_prepare(feature, weight, label)
    res = run_bass_kernel_spmd(nc, in_maps, list(range(N_CORES)))
    return _host_combine(res.results)


def run_sim(feature, weight, label, core=7, trace=False):
    """Simulate a single core and return its raw output tile + in_maps."""
    from concourse.bass_interp import MultiCoreSim

    nc = _get_program()
    in_maps = _host_prepare(feature, weight, label)
    sim = MultiCoreSim(nc, 1, trace=trace)
    for name, arr in in_maps[core].items():
        sim.cores[0].tensor(name)[:] = arr
    sim.simulate()
    return np.array(sim.cores[0].tensor("o")), in_maps


# revision 47
# speedup vs baseline: 1.0720x; 1.0720x over previous
"""CosFace (LMCL) loss + center loss, sharded over 8 Trainium2 NeuronCores.

Strategy (classification parallel over the class dim):
  - weight [50000,128] is zero-padded to [50176,128], split into 8 shards of
    6272 rows, and each shard is uploaded PRE-TRANSPOSED as bf16 wT [128,6272]
    (raw, unnormalized values - normalization happens on device).
  - On-device weight normalization: SQ = wT*wT (DVE, bf16 2x), per-class
    norms^2 via tiny PE matmuls (stationary = SQ tile, moving = ones column),
    1/norm via ACT Ln/Exp on a [128,49] tile, PE-transpose of the scale tile,
    SWDGE broadcast-DMA to replicate scales across partitions, then a fused
    multiply produces normalized wts (bf16).
  - Features are normalized in natural layout and PE-transposed into
    fT bf16 [128,1024] (built per 128-sample chunk for early pipelining).
  - Main loop: per chunk the 6272 local classes stream through PSUM in four
    fills (2048,2048,2048,128).  Fills 0-1 are consumed by ScalarE:
    exp(s*cos - 30) fused with accumulation (accum_out).  Fills 2-3 are
    consumed by VectorE using a Schraudolph bit-trick exp: one
    scalar_tensor_tensor computes int16(cos*K1 + K2) (the bf16 bit pattern of
    ~exp(s*cos - 30)), then one tensor_tensor_reduce on the bitcast values
    accumulates them at 2x rate.  This splits the 6.4M-element exp between
    the two engines.
  - The center-loss/target-cosine path (t, q) is sharded: each core handles
    only its own 128 samples from host-gathered fmy/wl tiles (Pool engine).
  - Host combines in float64: sums partial exp-sums across cores, subtracts
    the exact padding contribution, applies the CosFace margin correction,
    and assembles loss = mean(lse - s*(t-m)) + lambda*0.5*sum(q).
"""

import math

import ml_dtypes
import numpy as np

import concourse.bass as bass
import concourse.mybir as mybir
import concourse.tile as tile
from concourse.bass_utils import run_bass_kernel_spmd
from concourse.masks import make_identity

# ---------------------------------------------------------------------------
# Workaround for this container's walrus build: instructions carrying more
# than one semaphore wait fail codegen.  Move all but one wait onto
# standalone single-wait EventSemaphore instructions inserted immediately
# before, on the same engine.
# ---------------------------------------------------------------------------


def _split_multi_waits(nc):
    for fn in nc.m.functions:
        for bb in fn.blocks:
            insts = bb.instructions
            out = []
            changed = False
            for inst in insts:
                si = inst.sync_info
                if si is not None and len(si.on_wait) > 1:
                    waits = list(si.on_wait)
                    for w in waits[:-1]:
                        ev = mybir.InstEventSemaphore(
                            name=nc.get_next_instruction_name(), ins=[], outs=[]
                        )
                        ev.engine = inst.engine
                        ev.sync_info = mybir.SyncInfo(on_wait=[w], on_update=[])
                        nc.register_instruction(ev, overwrite=True)
                        out.append(ev)
                    si.on_wait[:] = waits[-1:]
                    changed = True
                out.append(inst)
            if changed:
                bb.instructions = out

# ---------------------------------------------------------------------------

F32 = mybir.dt.float32
BF16 = mybir.dt.bfloat16
I16 = mybir.dt.int16
AF = mybir.ActivationFunctionType
AX = mybir.AxisListType
OP = mybir.AluOpType

N_CORES = 8
N = 1024
C = 50000
D = 128
P = 128
NCH = N // P  # 8 sample chunks
CT = 49  # class tiles per core
CLOC = CT * P  # 6272 local classes
CPAD = N_CORES * CLOC  # 50176
NPAD = CPAD - C  # 176 zero rows (all on the last core)

S_SCALE = 30.0
M_MARGIN = 0.35
LAMBDA = 0.01
EXP_BIAS = -30.0  # exp(s*cos + EXP_BIAS); s*cos <= 30 so sums stay in fp32
EPS2 = 1e-16  # matches torch CosineSimilarity eps=1e-8 on squared norms

# Schraudolph bf16 exp: bitpattern(e^z) ~ int16(z*(2^7/ln2) + 2^7*(127-c)).
SCH_A = 128.0 / math.log(2.0)  # 184.6650
SCH_C = 0.0430
K1S = SCH_A * S_SCALE  # applied to cos directly
K2P = 128.0 * (127.0 - SCH_C) + SCH_A * EXP_BIAS  # folds the -30 bias


def _schraudolph_exp_np(cos):
    """Host replica of the device Schraudolph path (for the pad correction)."""
    i = np.round(np.float32(cos) * np.float32(K1S) + np.float32(K2P))
    return np.asarray(i.astype(np.int16).view(ml_dtypes.bfloat16), np.float64)


# class blocks: columns [o0, o1) and the engine that consumes each block.
# Pass 1 consumes blocks 0 (ACT) + 1 (DVE); pass 2 blocks 2 (ACT) + 3 (DVE).
# Sizes balance ACT (1 col/cycle @1.2GHz + accum fixed) against the DVE
# Schraudolph path (1x affine from PSUM + 4x accumulate).
BLOCKS = [(0, 2048), (2048, 3584), (3584, 5248), (5248, 6272)]
ACT_BLOCKS = (0, 2)
DVE_BLOCKS = (1, 3)
PASS_BLOCKS = [(0, 1), (2, 3)]
# norm-prep group g covers the blocks of pass g (prep of group 1 is emitted
# after pass 1's fills so it overlaps execution).
NORM_GROUPS = [(0, 1), (2, 3)]
MMB = 512  # matmul moving-block width (bass cap)


def _build_program(loop_iters=None):
    nc = bass.Bass(
        "TRN2", target_bir_lowering=False, debug=False, num_devices=N_CORES
    )
    wt_d = nc.dram_tensor("wt", [D, CLOC], BF16, kind="ExternalInput").ap()
    ft_d = nc.dram_tensor("ft", [D, N], BF16, kind="ExternalInput").ap()
    f_d = nc.dram_tensor("f", [N, D], F32, kind="ExternalInput").ap()
    fmy_d = nc.dram_tensor("fmy", [P, D], F32, kind="ExternalInput").ap()
    wl_d = nc.dram_tensor("wl", [P, D], F32, kind="ExternalInput").ap()
    rt_d = nc.dram_tensor("rt_scratch", [4, 16, P], BF16, kind="Internal").ap()
    rf_d = nc.dram_tensor("rf_scratch", [NCH, P], BF16, kind="Internal").ap()
    NOUT = 4 * NCH + 2  # 2 ACT accs + 2 DVE accs per chunk, then t, q
    o_d = nc.dram_tensor("o", [P, NOUT], F32, kind="ExternalOutput").ap()

    with tile.TileContext(nc) as tc:
        from contextlib import ExitStack, nullcontext

        with ExitStack() as ctx:
            cpool = ctx.enter_context(tc.tile_pool(name="const", bufs=1))
            big = ctx.enter_context(tc.tile_pool(name="big", bufs=1))
            st = ctx.enter_context(tc.tile_pool(name="stats", bufs=1))
            scr = ctx.enter_context(tc.tile_pool(name="scr", bufs=2))
            psum = ctx.enter_context(
                tc.tile_pool(name="psum", bufs=2, space="PSUM")
            )

            # ---- constants (outside the timing loop) ---------------------
            ident = cpool.tile([P, P], F32, tag="ident")
            make_identity(nc, ident[:])
            ebias = cpool.tile([P, 1], F32, tag="ebias")
            nc.gpsimd.memset(ebias[:], EXP_BIAS)
            ones_col = cpool.tile([P, 1], BF16, tag="ones_col")
            nc.gpsimd.memset(ones_col[:], 1.0)
            dummy = cpool.tile([P, 1], F32, tag="dummy")

            # ---- big tiles ------------------------------------------------
            wt = big.tile([P, CLOC], BF16, tag="wt")  # raw transposed weights
            sq = big.tile([P, CLOC], BF16, tag="sq")  # squares
            wts = big.tile([P, CLOC], BF16, tag="wts")  # normalized weights
            bsc = big.tile([P, CLOC], BF16, tag="bsc")  # broadcast scales
            f_nat = big.tile([P, N], F32, tag="f_nat")
            fsq = big.tile([P, N], F32, tag="fsq")
            fT = big.tile([P, N], BF16, tag="fT")  # raw transposed features
            fTs = big.tile([P, N], BF16, tag="fTs")  # normalized
            bscf = big.tile([P, N], BF16, tag="bscf")

            # ---- stats / small tiles -------------------------------------
            fns = st.tile([P, NCH], F32, tag="fns")
            fln = st.tile([P, NCH], F32, tag="fln")
            rf = st.tile([P, NCH], F32, tag="rf")
            ns = st.tile([P, 64], F32, tag="ns")  # norms^2, grouped cols
            rw = st.tile([P, 64], F32, tag="rw")  # 1/norm, grouped cols
            rT = st.tile([P, 5 * P], BF16, tag="rT")  # transposed scales
            fmy = st.tile([P, D], F32, tag="fmy")
            wl = st.tile([P, D], F32, tag="wl")
            pq = st.tile([P, 5 * D], F32, tag="pq")  # wl-path scratch
            nrm2 = st.tile([P, 2], F32, tag="nrm2")
            nrml = st.tile([P, 2], F32, tag="nrml")
            rfw = st.tile([P, 2], F32, tag="rfw")
            dots = st.tile([P, 1], F32, tag="dots")
            tt = st.tile([P, 1], F32, tag="tt")
            outt = st.tile([P, NOUT], F32, tag="outt")

            # DVE-path scratch (double-buffered via pool)
            eint = [
                scr.tile([P, 2048], I16, tag="eint", name=f"eint{i}")
                for i in range(2)
            ]
            edum = [
                scr.tile([P, 2048], BF16, tag="edum", name=f"edum{i}")
                for i in range(2)
            ]

            f3d_dram = f_d.rearrange("(c p) d -> p c d", p=P)
            f_nat3 = f_nat[:].rearrange("p (c d) -> p c d", d=D)
            sq3f = lambda ap: ap.rearrange("p (c d) -> p c d", d=D)

            # ---- ACT table warm ------------------------------------------
            nc.scalar.activation(dummy[:], ebias[:], AF.Exp)

            loop_cm = (
                tc.For_i(
                    0,
                    loop_iters,
                    1,
                    hint_engines=(
                        mybir.EngineType.PE,
                        mybir.EngineType.Activation,
                        mybir.EngineType.DVE,
                    ),
                )
                if loop_iters is not None
                else nullcontext()
            )
            _ = loop_cm.__enter__()

            # ---- input DMAs (two HWDGE rings: sync=SP, scalar=ACT) --------
            # sync ring carries the pass-1-critical loads + the scale
            # bounce/broadcast; scalar ring carries the rest in parallel.
            nc.sync.dma_start(fT[:], ft_d[:])
            for bi in (0, 1):
                o0, o1 = BLOCKS[bi]
                nc.sync.dma_start(wt[:, o0:o1], wt_d[:, o0:o1])
            for bi in (2, 3):
                o0, o1 = BLOCKS[bi]
                nc.scalar.dma_start(wt[:, o0:o1], wt_d[:, o0:o1])
            nc.scalar.dma_start(f_nat3[:], f3d_dram[:])
            nc.scalar.dma_start(fmy[:], fmy_d[:])
            nc.scalar.dma_start(wl[:], wl_d[:])

            # ---- feature stats; fTs = fT * broadcast(1/|f|) ---------------
            nc.vector.tensor_mul(fsq[:], f_nat[:], f_nat[:])
            nc.vector.reduce_sum(fns[:], sq3f(fsq[:]), axis=AX.X)
            nc.vector.tensor_scalar_max(fns[:], fns[:], EPS2)
            nc.scalar.activation(fln[:], fns[:], AF.Ln)
            nc.scalar.activation(rf[:], fln[:], AF.Exp, scale=-0.5)
            prf = psum.tile([P, 2048], F32, tag="ps")
            nc.tensor.transpose(prf[:NCH, :P], rf[:], ident[:])
            nc.vector.tensor_copy(rT[:NCH, 4 * P : 5 * P], prf[:NCH, :P])
            nc.sync.dma_start(rf_d[:, :], rT[:NCH, 4 * P : 5 * P])
            srcf = rf_d[:, :].unsqueeze(0).broadcast_to((P, NCH, P))
            dstf = bscf[:].rearrange("p (t c) -> p t c", c=P)
            nc.scalar.dma_start(dstf, srcf)
            nc.vector.tensor_mul(fTs[:], fT[:], bscf[:])

            # ---- weight norms, banded in two groups ----------------------
            def w_norm_group(g):
                blocks = NORM_GROUPS[g]
                base = 32 * g
                for bi in blocks:
                    o0, o1 = BLOCKS[bi]
                    eng = nc.vector if bi == 0 else nc.gpsimd
                    eng.tensor_mul(
                        sq[:, o0:o1], wt[:, o0:o1], wt[:, o0:o1]
                    )
                pns = psum.tile([P, 2048], F32, tag="ps")
                cols_of = {}
                col = 0
                for bi in blocks:
                    o0, o1 = BLOCKS[bi]
                    nt = (o1 - o0) // P
                    cols_of[bi] = (col, col + nt)
                    for t in range(nt):
                        nc.tensor.matmul(
                            pns[:, col : col + 1],
                            sq[:, o0 + t * P : o0 + (t + 1) * P],
                            ones_col[:],
                            start=True,
                            stop=True,
                        )
                        col += 1
                nc.vector.tensor_copy(ns[:, base : base + col], pns[:, :col])
                nc.vector.tensor_scalar_max(
                    ns[:, base : base + col], ns[:, base : base + col], EPS2
                )
                nc.scalar.activation(
                    rw[:, base : base + col], ns[:, base : base + col], AF.Ln
                )
                nc.scalar.activation(
                    rw[:, base : base + col],
                    rw[:, base : base + col],
                    AF.Exp,
                    scale=-0.5,
                )
                prt = psum.tile([P, 2048], F32, tag="ps")
                for bi in blocks:
                    c0, c1 = cols_of[bi]
                    nt = c1 - c0
                    nc.tensor.transpose(
                        prt[:nt, bi * P : bi * P + P],
                        rw[:, base + c0 : base + c1],
                        ident[:],
                    )
                    nc.vector.tensor_copy(
                        rT[:nt, bi * P : bi * P + P],
                        prt[:nt, bi * P : bi * P + P],
                    )
                    # replicate across partitions via a DRAM bounce (HWDGE)
                    nc.sync.dma_start(
                        rt_d[bi, :nt, :], rT[:nt, bi * P : bi * P + P]
                    )
                    o0, o1 = BLOCKS[bi]
                    src = (
                        rt_d[bi, :nt, :].unsqueeze(0).broadcast_to((P, nt, P))
                    )
                    dst = bsc[:, o0:o1].rearrange("p (t c) -> p t c", c=P)
                    nc.scalar.dma_start(dst, src)

            def w_scale(bi):
                o0, o1 = BLOCKS[bi]
                nc.vector.tensor_mul(
                    wts[:, o0:o1], wt[:, o0:o1], bsc[:, o0:o1]
                )

            w_norm_group(0)
            w_scale(0)
            w_scale(1)
            w_norm_group(1)

            # ---- main loop ------------------------------------------------
            def fill_and_consume(ch, bi, sidx):
                o0, o1 = BLOCKS[bi]
                gw = o1 - o0
                pt = psum.tile([P, 2048], F32, tag="ps")
                lhs = fTs[:, ch * P : (ch + 1) * P]
                for b in range(0, gw, MMB):
                    bw = min(MMB, gw - b)
                    nc.tensor.matmul(
                        pt[:, b : b + bw],
                        lhs,
                        wts[:, o0 + b : o0 + b + bw],
                        start=True,
                        stop=True,
                    )
                acc = outt[:, 4 * ch + bi : 4 * ch + bi + 1]
                if bi in ACT_BLOCKS:
                    nc.scalar.activation(
                        pt[:, :gw],
                        pt[:, :gw],
                        AF.Exp,
                        bias=ebias[:],
                        scale=S_SCALE,
                        accum_out=acc,
                    )
                else:
                    ei = eint[sidx][:, :gw]
                    ed = edum[sidx][:, :gw]
                    nc.vector.tensor_scalar(
                        ei, pt[:, :gw], K1S, K2P, OP.mult, OP.add
                    )
                    ev = ei.bitcast(BF16)
                    nc.vector.tensor_scalar(
                        ed, ev, 1.0, 0.0, OP.mult, OP.add, accum_out=acc
                    )

            # pass 1: blocks 0 (ACT) and 1 (DVE) for all chunks; the pass-2
            # scale multiplies are slipped in early so they finish under it
            for ch in range(NCH):
                fill_and_consume(ch, 0, 0)
                fill_and_consume(ch, 1, ch % 2)
                if ch == 1:
                    w_scale(2)
                elif ch == 3:
                    w_scale(3)

            # ---- wl-path (own 128 samples only) --------------------------
            pq5 = pq[:].rearrange("p (k d) -> p k d", d=D)
            nc.vector.tensor_mul(pq5[:, 0, :], fmy[:], fmy[:])
            nc.vector.tensor_mul(pq5[:, 1, :], wl[:], wl[:])
            nc.vector.reduce_sum(nrm2[:], pq5[:, 0:2, :], axis=AX.X)
            nc.vector.tensor_scalar_max(nrm2[:], nrm2[:], EPS2)
            nc.scalar.activation(nrml[:], nrm2[:], AF.Ln)
            nc.scalar.activation(rfw[:], nrml[:], AF.Exp, scale=-0.5)
            nc.vector.tensor_mul(pq5[:, 2, :], fmy[:], wl[:])
            nc.vector.reduce_sum(dots[:], pq5[:, 2:3, :], axis=AX.X)
            nc.vector.tensor_sub(pq5[:, 3, :], fmy[:], wl[:])
            nc.vector.tensor_mul(pq5[:, 4, :], pq5[:, 3, :], pq5[:, 3, :])
            nc.vector.reduce_sum(
                outt[:, 4 * NCH + 1 : 4 * NCH + 2], pq5[:, 4:5, :], axis=AX.X
            )
            nc.vector.tensor_mul(tt[:], dots[:], rfw[:, 0:1])
            nc.vector.tensor_mul(
                outt[:, 4 * NCH : 4 * NCH + 1], tt[:], rfw[:, 1:2]
            )

            # pass 2: blocks 2 (ACT) and 3 (DVE) for all chunks
            for ch in range(NCH):
                fill_and_consume(ch, 2, 0)
                fill_and_consume(ch, 3, ch % 2)

            nc.sync.dma_start(o_d[:], outt[:])

            loop_cm.__exit__(None, None, None)

    _split_multi_waits(nc)
    return nc


_NC_CACHE = None


def _get_program():
    global _NC_CACHE
    if _NC_CACHE is None:
        _NC_CACHE = _build_program()
    return _NC_CACHE


def _build_program_loop(iters):
    return _build_program(loop_iters=iters)


def _host_prepare(feature, weight, label):
    feature = np.ascontiguousarray(np.asarray(feature, dtype=np.float32))
    weight = np.asarray(weight, dtype=np.float32)
    label = np.asarray(label).astype(np.int64)
    wl = np.ascontiguousarray(weight[label])
    w_pad = np.zeros((CPAD, D), dtype=np.float32)
    w_pad[:C] = weight
    in_maps = []
    ft = np.ascontiguousarray(feature.T.astype(ml_dtypes.bfloat16))
    for k in range(N_CORES):
        shard = w_pad[k * CLOC : (k + 1) * CLOC]
        wt = np.ascontiguousarray(shard.T.astype(ml_dtypes.bfloat16))
        in_maps.append(
            {
                "wt": wt,
                "ft": ft,
                "f": feature,
                "fmy": np.ascontiguousarray(feature[k * P : (k + 1) * P]),
                "wl": np.ascontiguousarray(wl[k * P : (k + 1) * P]),
            }
        )
    return in_maps


def _host_combine(results):
    # device layout: out[p, col]; sample n = ch*128 + p
    outs = [np.asarray(r["o"], dtype=np.float64) for r in results]
    A = sum(o[:, : 4 * NCH].reshape(P, NCH, 4).sum(axis=2) for o in outs)
    A_n = A.T.reshape(N)  # [1024]
    t_n = np.concatenate([o[:, 4 * NCH] for o in outs])  # [1024]
    q_n = np.concatenate([o[:, 4 * NCH + 1] for o in outs])

    # padded classes live in the DVE (Schraudolph) blocks of the last core:
    # each contributes the exact bit-trick value of exp(s*0 - 30).
    pad_term = _schraudolph_exp_np(np.zeros(1))[0]
    S_raw = A_n - NPAD * pad_term
    S_fix = (
        S_raw
        - np.exp(S_SCALE * t_n + EXP_BIAS)
        + np.exp(S_SCALE * (t_n - M_MARGIN) + EXP_BIAS)
    )
    lse = np.log(S_fix) - EXP_BIAS
    target_logit = S_SCALE * (t_n - M_MARGIN)
    loss_lmc = np.mean(lse - target_logit)
    loss_c = 0.5 * np.sum(q_n)
    return np.float32(loss_lmc + LAMBDA * loss_c)


def kernel(feature, weight, label):
    nc = _get_program()
    in_maps = _host_prepare(feature, weight, label)
    res = run_bass_kernel_spmd(nc, in_maps, list(range(N_CORES)))
    return _host_combine(res.results)


def run_sim(feature, weight, label, core=7, trace=False):
    """Simulate a single core and return its raw output tile + in_maps."""
    from concourse.bass_interp import MultiCoreSim

    nc = _get_program()
    in_maps = _host_prepare(feature, weight, label)
    sim = MultiCoreSim(nc, 1, trace=trace)
    for name, arr in in_maps[core].items():
        sim.cores[0].tensor(name)[:] = arr
    sim.simulate()
    return np.array(sim.cores[0].tensor("o")), in_maps


# revision 48
# speedup vs baseline: 1.1123x; 1.0376x over previous
"""CosFace (LMCL) loss + center loss, sharded over 8 Trainium2 NeuronCores.

Strategy (classification parallel over the class dim):
  - weight [50000,128] is zero-padded to [50176,128], split into 8 shards of
    6272 rows, and each shard is uploaded PRE-TRANSPOSED as bf16 wT [128,6272]
    (raw, unnormalized values - normalization happens on device).
  - On-device weight normalization: SQ = wT*wT (DVE, bf16 2x), per-class
    norms^2 via tiny PE matmuls (stationary = SQ tile, moving = ones column),
    1/norm via ACT Ln/Exp on a small per-partition tile, PE-transpose of the
    scale tile, a DRAM-bounce broadcast DMA to replicate scales across
    partitions, then a fused bf16 multiply produces normalized wts.
  - Features are normalized in natural layout and PE-transposed into
    fT bf16 [128,1024].
  - Main loop: per 128-sample chunk the 6272 local classes stream through
    PSUM in four fills (2048,2048,2048,128).  Two fills go to ScalarE:
    exp(s*cos - 30) fused with accumulation (accum_out).  The other two go
    to VectorE using a Schraudolph bit-trick exp: one tensor_scalar computes
    int16(cos*K1 + K2) (the bf16 bit pattern of ~exp(s*cos - 30)), then a
    second tensor_scalar on the bitcast values accumulates them at 4x rate.
    This splits the 6.4M-element exp between the two engines.
  - The center-loss/target-cosine path (t, q) is sharded: each core handles
    only its own 128 samples from host-gathered fmy/wl tiles.
  - Host combines in float64: sums partial exp-sums across cores, subtracts
    the exact padding contribution, applies the CosFace margin correction,
    and assembles loss = mean(lse - s*(t-m)) + lambda*0.5*sum(q).
"""

import math

import ml_dtypes
import numpy as np

import concourse.bass as bass
import concourse.mybir as mybir
import concourse.tile as tile
from concourse.bass_utils import run_bass_kernel_spmd
from concourse.masks import make_identity

# ---------------------------------------------------------------------------
# Workaround for this container's walrus build: instructions carrying more
# than one semaphore wait fail codegen.  Move all but one wait onto
# standalone single-wait EventSemaphore instructions inserted immediately
# before, on the same engine.
# ---------------------------------------------------------------------------


def _split_multi_waits(nc):
    for fn in nc.m.functions:
        for bb in fn.blocks:
            insts = bb.instructions
            out = []
            changed = False
            for inst in insts:
                si = inst.sync_info
                if si is not None and len(si.on_wait) > 1:
                    waits = list(si.on_wait)
                    for w in waits[:-1]:
                        ev = mybir.InstEventSemaphore(
                            name=nc.get_next_instruction_name(), ins=[], outs=[]
                        )
                        ev.engine = inst.engine
                        ev.sync_info = mybir.SyncInfo(on_wait=[w], on_update=[])
                        nc.register_instruction(ev, overwrite=True)
                        out.append(ev)
                    si.on_wait[:] = waits[-1:]
                    changed = True
                out.append(inst)
            if changed:
                bb.instructions = out

# ---------------------------------------------------------------------------

F32 = mybir.dt.float32
BF16 = mybir.dt.bfloat16
I16 = mybir.dt.int16
AF = mybir.ActivationFunctionType
AX = mybir.AxisListType
OP = mybir.AluOpType

N_CORES = 8
N = 1024
C = 50000
D = 128
P = 128
NCH = N // P  # 8 sample chunks
CT = 49  # class tiles per core
CLOC = CT * P  # 6272 local classes
CPAD = N_CORES * CLOC  # 50176
NPAD = CPAD - C  # 176 zero rows (all on the last core)

S_SCALE = 30.0
M_MARGIN = 0.35
LAMBDA = 0.01
EXP_BIAS = -30.0  # exp(s*cos + EXP_BIAS); s*cos <= 30 so sums stay in fp32
EPS2 = 1e-16  # matches torch CosineSimilarity eps=1e-8 on squared norms

# Schraudolph bf16 exp: bitpattern(e^z) ~ int16(z*(2^7/ln2) + 2^7*(127-c)).
SCH_A = 128.0 / math.log(2.0)  # 184.6650
SCH_C = 0.0430
K1S = SCH_A * S_SCALE  # applied to cos directly
K2P = 128.0 * (127.0 - SCH_C) + SCH_A * EXP_BIAS  # folds the -30 bias


def _schraudolph_exp_np(cos):
    """Host replica of the device Schraudolph path (for the pad correction)."""
    i = np.round(np.float32(cos) * np.float32(K1S) + np.float32(K2P))
    return np.asarray(i.astype(np.int16).view(ml_dtypes.bfloat16), np.float64)


# class blocks: columns [o0, o1) and the engine that consumes each block.
# Blocks 0-1 -> ACT exp path, blocks 2-3 -> DVE Schraudolph path.
BLOCKS = [(0, 2048), (2048, 4096), (4096, 6144), (6144, 6272)]
ACT_BLOCKS = (0, 1)
DVE_BLOCKS = (2, 3)
# norms prepped in two groups interleaving ACT/DVE blocks so pass 1 (blocks
# 0 and 2) can start after group 0.
NORM_GROUPS = [(0, 2), (1, 3)]
MMB = 512  # matmul moving-block width (bass cap)


def _build_program(loop_iters=None):
    nc = bass.Bass(
        "TRN2", target_bir_lowering=False, debug=False, num_devices=N_CORES
    )
    wt_d = nc.dram_tensor("wt", [D, CLOC], BF16, kind="ExternalInput").ap()
    f_d = nc.dram_tensor("f", [N, D], F32, kind="ExternalInput").ap()
    fmy_d = nc.dram_tensor("fmy", [P, D], F32, kind="ExternalInput").ap()
    wl_d = nc.dram_tensor("wl", [P, D], F32, kind="ExternalInput").ap()
    rt_d = nc.dram_tensor("rt_scratch", [4, 16, P], BF16, kind="Internal").ap()
    NOUT = 4 * NCH + 2  # 2 ACT accs + 2 DVE accs per chunk, then t, q
    o_d = nc.dram_tensor("o", [P, NOUT], F32, kind="ExternalOutput").ap()

    with tile.TileContext(nc) as tc:
        from contextlib import ExitStack, nullcontext

        with ExitStack() as ctx:
            cpool = ctx.enter_context(tc.tile_pool(name="const", bufs=1))
            big = ctx.enter_context(tc.tile_pool(name="big", bufs=1))
            st = ctx.enter_context(tc.tile_pool(name="stats", bufs=1))
            scr = ctx.enter_context(tc.tile_pool(name="scr", bufs=2))
            psum = ctx.enter_context(
                tc.tile_pool(name="psum", bufs=2, space="PSUM")
            )

            # ---- constants (outside the timing loop) ---------------------
            ident = cpool.tile([P, P], F32, tag="ident")
            make_identity(nc, ident[:])
            ebias = cpool.tile([P, 1], F32, tag="ebias")
            nc.gpsimd.memset(ebias[:], EXP_BIAS)
            ones_col = cpool.tile([P, 1], BF16, tag="ones_col")
            nc.gpsimd.memset(ones_col[:], 1.0)
            dummy = cpool.tile([P, 1], F32, tag="dummy")

            # ---- big tiles ------------------------------------------------
            wt = big.tile([P, CLOC], BF16, tag="wt")  # raw transposed weights
            sq = big.tile([P, CLOC], BF16, tag="sq")  # squares
            wts = big.tile([P, CLOC], BF16, tag="wts")  # normalized weights
            bsc = big.tile([P, CLOC], BF16, tag="bsc")  # broadcast scales
            f_nat = big.tile([P, N], F32, tag="f_nat")
            fhat = big.tile([P, N], F32, tag="fhat")
            fT = big.tile([P, N], BF16, tag="fT")

            # ---- stats / small tiles -------------------------------------
            fns = st.tile([P, NCH], F32, tag="fns")
            fln = st.tile([P, NCH], F32, tag="fln")
            rf = st.tile([P, NCH], F32, tag="rf")
            ns = st.tile([P, 64], F32, tag="ns")  # norms^2, grouped cols
            rw = st.tile([P, 64], F32, tag="rw")  # 1/norm, grouped cols
            rT = st.tile([P, 4 * P], BF16, tag="rT")  # transposed scales
            fmy = st.tile([P, D], F32, tag="fmy")
            wl = st.tile([P, D], F32, tag="wl")
            pq = st.tile([P, 5 * D], F32, tag="pq")  # wl-path scratch
            nrm2 = st.tile([P, 2], F32, tag="nrm2")
            nrml = st.tile([P, 2], F32, tag="nrml")
            rfw = st.tile([P, 2], F32, tag="rfw")
            dots = st.tile([P, 1], F32, tag="dots")
            tt = st.tile([P, 1], F32, tag="tt")
            outt = st.tile([P, NOUT], F32, tag="outt")

            # DVE-path scratch (double-buffered via pool)
            eint = [
                scr.tile([P, 2048], I16, tag="eint", name=f"eint{i}")
                for i in range(2)
            ]
            edum = [
                scr.tile([P, 2048], BF16, tag="edum", name=f"edum{i}")
                for i in range(2)
            ]

            f3d_dram = f_d.rearrange("(c p) d -> p c d", p=P)
            f_nat3 = f_nat[:].rearrange("p (c d) -> p c d", d=D)
            fhat3 = fhat[:].rearrange("p (c d) -> p c d", d=D)
            sq3f = lambda ap: ap.rearrange("p (c d) -> p c d", d=D)

            # ---- ACT table warm ------------------------------------------
            nc.scalar.activation(dummy[:], ebias[:], AF.Exp)

            loop_cm = (
                tc.For_i(
                    0,
                    loop_iters,
                    1,
                    hint_engines=(
                        mybir.EngineType.PE,
                        mybir.EngineType.Activation,
                        mybir.EngineType.DVE,
                    ),
                )
                if loop_iters is not None
                else nullcontext()
            )
            _ = loop_cm.__enter__()

            # ---- input DMAs (one HWDGE ring, issue order = priority) -----
            nc.sync.dma_start(f_nat3[:], f3d_dram[:])
            for bi in (0, 2, 1, 3):
                o0, o1 = BLOCKS[bi]
                nc.sync.dma_start(wt[:, o0:o1], wt_d[:, o0:o1])
            nc.sync.dma_start(fmy[:], fmy_d[:])
            nc.sync.dma_start(wl[:], wl_d[:])

            # ---- feature stats + fT --------------------------------------
            nc.vector.tensor_mul(fhat[:], f_nat[:], f_nat[:])
            nc.vector.reduce_sum(fns[:], sq3f(fhat[:]), axis=AX.X)
            nc.vector.tensor_scalar_max(fns[:], fns[:], EPS2)
            nc.scalar.activation(fln[:], fns[:], AF.Ln)
            nc.scalar.activation(rf[:], fln[:], AF.Exp, scale=-0.5)
            rfb = rf[:].unsqueeze(2).broadcast_to((P, NCH, D))
            nc.vector.tensor_mul(fhat3[:], f_nat3[:], rfb)
            for h in range(2):
                ptf = psum.tile([P, 2048], F32, tag="ps")
                for j in range(4):
                    ch = 4 * h + j
                    nc.tensor.transpose(
                        ptf[:, j * P : (j + 1) * P], fhat3[:, ch, :], ident[:]
                    )
                nc.vector.tensor_copy(
                    fT[:, h * 512 : (h + 1) * 512], ptf[:, :512]
                )

            # ---- weight norms + scales, in two groups --------------------
            def w_prep_group(g):
                blocks = NORM_GROUPS[g]
                base = 32 * g
                for bi in blocks:
                    o0, o1 = BLOCKS[bi]
                    nc.vector.tensor_mul(
                        sq[:, o0:o1], wt[:, o0:o1], wt[:, o0:o1]
                    )
                pns = psum.tile([P, 2048], F32, tag="ps")
                col = 0
                cols_of = {}
                for bi in blocks:
                    o0, o1 = BLOCKS[bi]
                    nt = (o1 - o0) // P
                    cols_of[bi] = (col, col + nt)
                    for t in range(nt):
                        nc.tensor.matmul(
                            pns[:, col : col + 1],
                            sq[:, o0 + t * P : o0 + (t + 1) * P],
                            ones_col[:],
                            start=True,
                            stop=True,
                        )
                        col += 1
                nc.vector.tensor_copy(ns[:, base : base + col], pns[:, :col])
                nc.vector.tensor_scalar_max(
                    ns[:, base : base + col], ns[:, base : base + col], EPS2
                )
                nc.scalar.activation(
                    rw[:, base : base + col], ns[:, base : base + col], AF.Ln
                )
                nc.scalar.activation(
                    rw[:, base : base + col],
                    rw[:, base : base + col],
                    AF.Exp,
                    scale=-0.5,
                )
                for bi in blocks:
                    c0, c1 = cols_of[bi]
                    o0, o1 = BLOCKS[bi]
                    nt = c1 - c0
                    prt = psum.tile([P, 2048], F32, tag="ps")
                    nc.tensor.transpose(
                        prt[:nt, :P], rw[:, base + c0 : base + c1], ident[:]
                    )
                    nc.vector.tensor_copy(
                        rT[:nt, bi * P : bi * P + P], prt[:nt, :P]
                    )
                    # replicate scale rows across all 128 partitions via a
                    # DRAM bounce (HWDGE both ways, dtypes match)
                    nc.sync.dma_start(
                        rt_d[bi, :nt, :], rT[:nt, bi * P : bi * P + P]
                    )
                    src = rt_d[bi, :nt, :].unsqueeze(0).broadcast_to((P, nt, P))
                    dst = bsc[:, o0:o1].rearrange("p (t c) -> p t c", c=P)
                    nc.sync.dma_start(dst, src)
                    # wts = wt * bsc
                    nc.vector.tensor_mul(
                        wts[:, o0:o1], wt[:, o0:o1], bsc[:, o0:o1]
                    )

            w_prep_group(0)
            w_prep_group(1)

            # ---- main loop ------------------------------------------------
            def fill_and_consume(ch, bi, sidx):
                o0, o1 = BLOCKS[bi]
                gw = o1 - o0
                pt = psum.tile([P, 2048], F32, tag="ps")
                lhs = fT[:, ch * P : (ch + 1) * P]
                for b in range(0, gw, MMB):
                    bw = min(MMB, gw - b)
                    nc.tensor.matmul(
                        pt[:, b : b + bw],
                        lhs,
                        wts[:, o0 + b : o0 + b + bw],
                        start=True,
                        stop=True,
                    )
                acc = outt[:, 4 * ch + bi : 4 * ch + bi + 1]
                if bi in ACT_BLOCKS:
                    nc.scalar.activation(
                        pt[:, :gw],
                        pt[:, :gw],
                        AF.Exp,
                        bias=ebias[:],
                        scale=S_SCALE,
                        accum_out=acc,
                    )
                else:
                    ei = eint[sidx][:, :gw]
                    ed = edum[sidx][:, :gw]
                    nc.vector.tensor_scalar(
                        ei, pt[:, :gw], K1S, K2P, OP.mult, OP.add
                    )
                    ev = ei.bitcast(BF16)
                    nc.vector.tensor_scalar(
                        ed, ev, 1.0, 0.0, OP.mult, OP.add, accum_out=acc
                    )

            # pass 1: blocks 0 (ACT) and 2 (DVE) for all chunks
            for ch in range(NCH):
                fill_and_consume(ch, 0, 0)
                fill_and_consume(ch, 2, ch % 2)

            # ---- wl-path (own 128 samples only) --------------------------
            pq5 = pq[:].rearrange("p (k d) -> p k d", d=D)
            nc.vector.tensor_mul(pq5[:, 0, :], fmy[:], fmy[:])
            nc.vector.tensor_mul(pq5[:, 1, :], wl[:], wl[:])
            nc.vector.reduce_sum(nrm2[:], pq5[:, 0:2, :], axis=AX.X)
            nc.vector.tensor_scalar_max(nrm2[:], nrm2[:], EPS2)
            nc.scalar.activation(nrml[:], nrm2[:], AF.Ln)
            nc.scalar.activation(rfw[:], nrml[:], AF.Exp, scale=-0.5)
            nc.vector.tensor_mul(pq5[:, 2, :], fmy[:], wl[:])
            nc.vector.reduce_sum(dots[:], pq5[:, 2:3, :], axis=AX.X)
            nc.vector.tensor_sub(pq5[:, 3, :], fmy[:], wl[:])
            nc.vector.tensor_mul(pq5[:, 4, :], pq5[:, 3, :], pq5[:, 3, :])
            nc.vector.reduce_sum(
                outt[:, 4 * NCH + 1 : 4 * NCH + 2], pq5[:, 4:5, :], axis=AX.X
            )
            nc.vector.tensor_mul(tt[:], dots[:], rfw[:, 0:1])
            nc.vector.tensor_mul(
                outt[:, 4 * NCH : 4 * NCH + 1], tt[:], rfw[:, 1:2]
            )

            # pass 2: blocks 1 (ACT) and 3 (DVE) for all chunks
            for ch in range(NCH):
                fill_and_consume(ch, 1, 0)
                fill_and_consume(ch, 3, ch % 2)

            nc.sync.dma_start(o_d[:], outt[:])

            loop_cm.__exit__(None, None, None)

    _split_multi_waits(nc)
    return nc


_NC_CACHE = None


def _get_program():
    global _NC_CACHE
    if _NC_CACHE is None:
        _NC_CACHE = _build_program()
    return _NC_CACHE


def _build_program_loop(iters):
    return _build_program(loop_iters=iters)


def _host_prepare(feature, weight, label):
    feature = np.ascontiguousarray(np.asarray(feature, dtype=np.float32))
    weight = np.asarray(weight, dtype=np.float32)
    label = np.asarray(label).astype(np.int64)
    wl = np.ascontiguousarray(weight[label])
    w_pad = np.zeros((CPAD, D), dtype=np.float32)
    w_pad[:C] = weight
    in_maps = []
    for k in range(N_CORES):
        shard = w_pad[k * CLOC : (k + 1) * CLOC]
        wt = np.ascontiguousarray(shard.T.astype(ml_dtypes.bfloat16))
        in_maps.append(
            {
                "wt": wt,
                "f": feature,
                "fmy": np.ascontiguousarray(feature[k * P : (k + 1) * P]),
                "wl": np.ascontiguousarray(wl[k * P : (k + 1) * P]),
            }
        )
    return in_maps


def _host_combine(results):
    # device layout: out[p, col]; sample n = ch*128 + p
    outs = [np.asarray(r["o"], dtype=np.float64) for r in results]
    A = sum(o[:, : 4 * NCH].reshape(P, NCH, 4).sum(axis=2) for o in outs)
    A_n = A.T.reshape(N)  # [1024]
    t_n = np.concatenate([o[:, 4 * NCH] for o in outs])  # [1024]
    q_n = np.concatenate([o[:, 4 * NCH + 1] for o in outs])

    # padded classes live in the DVE (Schraudolph) blocks of the last core:
    # each contributes the exact bit-trick value of exp(s*0 - 30).
    pad_term = _schraudolph_exp_np(np.zeros(1))[0]
    S_raw = A_n - NPAD * pad_term
    S_fix = (
        S_raw
        - np.exp(S_SCALE * t_n + EXP_BIAS)
        + np.exp(S_SCALE * (t_n - M_MARGIN) + EXP_BIAS)
    )
    lse = np.log(S_fix) - EXP_BIAS
    target_logit = S_SCALE * (t_n - M_MARGIN)
    loss_lmc = np.mean(lse - target_logit)
    loss_c = 0.5 * np.sum(q_n)
    return np.float32(loss_lmc + LAMBDA * loss_c)


def kernel(feature, weight, label):
    nc = _get_program()
    in_maps = _host_prepare(feature, weight, label)
    res = run_bass_kernel_spmd(nc, in_maps, list(range(N_CORES)))
    return _host_combine(res.results)


def run_sim(feature, weight, label, core=7, trace=False):
    """Simulate a single core and return its raw output tile + in_maps."""
    from concourse.bass_interp import MultiCoreSim

    nc = _get_program()
    in_maps = _host_prepare(feature, weight, label)
    sim = MultiCoreSim(nc, 1, trace=trace)
    for name, arr in in_maps[core].items():
        sim.cores[0].tensor(name)[:] = arr
    sim.simulate()
    return np.array(sim.cores[0].tensor("o")), in_maps


# revision 50
# speedup vs baseline: 1.1647x; 1.0471x over previous
"""CosFace (LMCL) loss + center loss, sharded over 8 Trainium2 NeuronCores.

Strategy (classification parallel over the class dim):
  - weight [50000,128] is zero-padded to [50176,128], split into 8 shards of
    6272 rows, and each shard is uploaded PRE-TRANSPOSED as bf16 wT [128,6272]
    (raw, unnormalized values - normalization happens on device).
  - On-device weight normalization: SQ = wT*wT (DVE, bf16 2x), per-class
    norms^2 via tiny PE matmuls (stationary = SQ tile, moving = ones column),
    1/norm via ACT Ln/Exp on a small per-partition tile, PE-transpose of the
    scale tile, a DRAM-bounce broadcast DMA to replicate scales across
    partitions, then a fused bf16 multiply produces normalized wts.
  - Features are normalized in natural layout and PE-transposed into
    fT bf16 [128,1024].
  - Main loop: per 128-sample chunk the 6272 local classes stream through
    PSUM in four fills (2048,2048,2048,128).  Two fills go to ScalarE:
    exp(s*cos - 30) fused with accumulation (accum_out).  The other two go
    to VectorE using a Schraudolph bit-trick exp: one tensor_scalar computes
    int16(cos*K1 + K2) (the bf16 bit pattern of ~exp(s*cos - 30)), then a
    second tensor_scalar on the bitcast values accumulates them at 4x rate.
    This splits the 6.4M-element exp between the two engines.
  - The center-loss/target-cosine path (t, q) is sharded: each core handles
    only its own 128 samples from host-gathered fmy/wl tiles.
  - Host combines in float64: sums partial exp-sums across cores, subtracts
    the exact padding contribution, applies the CosFace margin correction,
    and assembles loss = mean(lse - s*(t-m)) + lambda*0.5*sum(q).
"""

import math

import ml_dtypes
import numpy as np

import concourse.bass as bass
import concourse.mybir as mybir
import concourse.tile as tile
from concourse.bass_utils import run_bass_kernel_spmd
from concourse.masks import make_identity

# ---------------------------------------------------------------------------
# Workaround for this container's walrus build: instructions carrying more
# than one semaphore wait fail codegen.  Move all but one wait onto
# standalone single-wait EventSemaphore instructions inserted immediately
# before, on the same engine.
# ---------------------------------------------------------------------------


def _split_multi_waits(nc):
    for fn in nc.m.functions:
        for bb in fn.blocks:
            insts = bb.instructions
            out = []
            changed = False
            for inst in insts:
                si = inst.sync_info
                if si is not None and len(si.on_wait) > 1:
                    waits = list(si.on_wait)
                    for w in waits[:-1]:
                        ev = mybir.InstEventSemaphore(
                            name=nc.get_next_instruction_name(), ins=[], outs=[]
                        )
                        ev.engine = inst.engine
                        ev.sync_info = mybir.SyncInfo(on_wait=[w], on_update=[])
                        nc.register_instruction(ev, overwrite=True)
                        out.append(ev)
                    si.on_wait[:] = waits[-1:]
                    changed = True
                out.append(inst)
            if changed:
                bb.instructions = out

# ---------------------------------------------------------------------------

F32 = mybir.dt.float32
BF16 = mybir.dt.bfloat16
I16 = mybir.dt.int16
AF = mybir.ActivationFunctionType
AX = mybir.AxisListType
OP = mybir.AluOpType

N_CORES = 8
N = 1024
C = 50000
D = 128
P = 128
NCH = N // P  # 8 sample chunks
CT = 49  # class tiles per core
CLOC = CT * P  # 6272 local classes
CPAD = N_CORES * CLOC  # 50176
NPAD = CPAD - C  # 176 zero rows (all on the last core)

S_SCALE = 30.0
M_MARGIN = 0.35
LAMBDA = 0.01
EXP_BIAS = -30.0  # exp(s*cos + EXP_BIAS); s*cos <= 30 so sums stay in fp32
EPS2 = 1e-16  # matches torch CosineSimilarity eps=1e-8 on squared norms

# Schraudolph bf16 exp: bitpattern(e^z) ~ int16(z*(2^7/ln2) + 2^7*(127-c)).
SCH_A = 128.0 / math.log(2.0)  # 184.6650
SCH_C = 0.0430
K1S = SCH_A * S_SCALE  # applied to cos directly
K2P = 128.0 * (127.0 - SCH_C) + SCH_A * EXP_BIAS  # folds the -30 bias


def _schraudolph_exp_np(cos):
    """Host replica of the device Schraudolph path (for the pad correction)."""
    i = np.round(np.float32(cos) * np.float32(K1S) + np.float32(K2P))
    return np.asarray(i.astype(np.int16).view(ml_dtypes.bfloat16), np.float64)


# class blocks: columns [o0, o1) and the engine that consumes each block.
# Blocks 0-1 -> ACT exp path, blocks 2-3 -> DVE Schraudolph path.
BLOCKS = [(0, 2048), (2048, 4096), (4096, 6144), (6144, 6272)]
ACT_BLOCKS = (0, 1)
DVE_BLOCKS = (2, 3)
# norms prepped in two groups interleaving ACT/DVE blocks so pass 1 (blocks
# 0 and 2) can start after group 0.
NORM_GROUPS = [(0, 2), (1, 3)]
MMB = 512  # matmul moving-block width (bass cap)


def _build_program(loop_iters=None):
    nc = bass.Bass(
        "TRN2", target_bir_lowering=False, debug=False, num_devices=N_CORES
    )
    wt_d = nc.dram_tensor("wt", [D, CLOC], BF16, kind="ExternalInput").ap()
    f_d = nc.dram_tensor("f", [N, D], F32, kind="ExternalInput").ap()
    fmy_d = nc.dram_tensor("fmy", [P, D], F32, kind="ExternalInput").ap()
    wl_d = nc.dram_tensor("wl", [P, D], F32, kind="ExternalInput").ap()
    rt_d = nc.dram_tensor("rt_scratch", [4, 16, P], BF16, kind="Internal").ap()
    NOUT = 4 * NCH + 2  # 2 ACT accs + 2 DVE accs per chunk, then t, q
    o_d = nc.dram_tensor("o", [P, NOUT], F32, kind="ExternalOutput").ap()

    with tile.TileContext(nc) as tc:
        from contextlib import ExitStack, nullcontext

        with ExitStack() as ctx:
            cpool = ctx.enter_context(tc.tile_pool(name="const", bufs=1))
            big = ctx.enter_context(tc.tile_pool(name="big", bufs=1))
            st = ctx.enter_context(tc.tile_pool(name="stats", bufs=1))
            scr = ctx.enter_context(tc.tile_pool(name="scr", bufs=2))
            psum = ctx.enter_context(
                tc.tile_pool(name="psum", bufs=2, space="PSUM")
            )

            # ---- constants (outside the timing loop) ---------------------
            ident = cpool.tile([P, P], F32, tag="ident")
            make_identity(nc, ident[:])
            ebias = cpool.tile([P, 1], F32, tag="ebias")
            nc.gpsimd.memset(ebias[:], EXP_BIAS)
            ones_col = cpool.tile([P, 1], BF16, tag="ones_col")
            nc.gpsimd.memset(ones_col[:], 1.0)
            dummy = cpool.tile([P, 1], F32, tag="dummy")

            # ---- big tiles ------------------------------------------------
            wt = big.tile([P, CLOC], BF16, tag="wt")  # raw transposed weights
            sq = big.tile([P, CLOC], BF16, tag="sq")  # squares
            wts = big.tile([P, CLOC], BF16, tag="wts")  # normalized weights
            bsc = big.tile([P, CLOC], BF16, tag="bsc")  # broadcast scales
            f_nat = big.tile([P, N], F32, tag="f_nat")
            fhat = big.tile([P, N], F32, tag="fhat")
            fT = big.tile([P, N], BF16, tag="fT")

            # ---- stats / small tiles -------------------------------------
            fns = st.tile([P, NCH], F32, tag="fns")
            fln = st.tile([P, NCH], F32, tag="fln")
            rf = st.tile([P, NCH], F32, tag="rf")
            ns = st.tile([P, 64], F32, tag="ns")  # norms^2, grouped cols
            rw = st.tile([P, 64], F32, tag="rw")  # 1/norm, grouped cols
            rT = st.tile([P, 4 * P], BF16, tag="rT")  # transposed scales
            fmy = st.tile([P, D], F32, tag="fmy")
            wl = st.tile([P, D], F32, tag="wl")
            pq = st.tile([P, 5 * D], F32, tag="pq")  # wl-path scratch
            nrm2 = st.tile([P, 2], F32, tag="nrm2")
            nrml = st.tile([P, 2], F32, tag="nrml")
            rfw = st.tile([P, 2], F32, tag="rfw")
            dots = st.tile([P, 1], F32, tag="dots")
            tt = st.tile([P, 1], F32, tag="tt")
            outt = st.tile([P, NOUT], F32, tag="outt")

            # DVE-path scratch (double-buffered via pool)
            eint = [
                scr.tile([P, 2048], I16, tag="eint", name=f"eint{i}")
                for i in range(2)
            ]
            edum = [
                scr.tile([P, 2048], BF16, tag="edum", name=f"edum{i}")
                for i in range(2)
            ]

            f3d_dram = f_d.rearrange("(c p) d -> p c d", p=P)
            f_nat3 = f_nat[:].rearrange("p (c d) -> p c d", d=D)
            fhat3 = fhat[:].rearrange("p (c d) -> p c d", d=D)
            sq3f = lambda ap: ap.rearrange("p (c d) -> p c d", d=D)

            # ---- ACT table warm ------------------------------------------
            nc.scalar.activation(dummy[:], ebias[:], AF.Exp)

            loop_cm = (
                tc.For_i(
                    0,
                    loop_iters,
                    1,
                    hint_engines=(
                        mybir.EngineType.PE,
                        mybir.EngineType.Activation,
                        mybir.EngineType.DVE,
                    ),
                )
                if loop_iters is not None
                else nullcontext()
            )
            _ = loop_cm.__enter__()

            # ---- input DMAs (one HWDGE ring, issue order = priority) -----
            nc.sync.dma_start(f_nat3[:], f3d_dram[:])
            for bi in (0, 2, 1, 3):
                o0, o1 = BLOCKS[bi]
                nc.sync.dma_start(wt[:, o0:o1], wt_d[:, o0:o1])
            nc.sync.dma_start(fmy[:], fmy_d[:])
            nc.sync.dma_start(wl[:], wl_d[:])

            # ---- feature stats + fT --------------------------------------
            nc.vector.tensor_mul(fhat[:], f_nat[:], f_nat[:])
            nc.vector.reduce_sum(fns[:], sq3f(fhat[:]), axis=AX.X)
            nc.vector.tensor_scalar_max(fns[:], fns[:], EPS2)
            nc.scalar.activation(fln[:], fns[:], AF.Ln)
            nc.scalar.activation(rf[:], fln[:], AF.Exp, scale=-0.5)
            rfb = rf[:].unsqueeze(2).broadcast_to((P, NCH, D))
            nc.vector.tensor_mul(fhat3[:], f_nat3[:], rfb)
            for h in range(2):
                ptf = psum.tile([P, 2048], F32, tag="ps")
                for j in range(4):
                    ch = 4 * h + j
                    nc.tensor.transpose(
                        ptf[:, j * P : (j + 1) * P], fhat3[:, ch, :], ident[:]
                    )
                nc.vector.tensor_copy(
                    fT[:, h * 512 : (h + 1) * 512], ptf[:, :512]
                )

            # ---- weight norms + scales, in two groups --------------------
            def w_prep_group(g):
                blocks = NORM_GROUPS[g]
                base = 32 * g
                for bi in blocks:
                    o0, o1 = BLOCKS[bi]
                    nc.vector.tensor_mul(
                        sq[:, o0:o1], wt[:, o0:o1], wt[:, o0:o1]
                    )
                pns = psum.tile([P, 2048], F32, tag="ps")
                col = 0
                cols_of = {}
                for bi in blocks:
                    o0, o1 = BLOCKS[bi]
                    nt = (o1 - o0) // P
                    cols_of[bi] = (col, col + nt)
                    for t in range(nt):
                        nc.tensor.matmul(
                            pns[:, col : col + 1],
                            sq[:, o0 + t * P : o0 + (t + 1) * P],
                            ones_col[:],
                            start=True,
                            stop=True,
                        )
                        col += 1
                nc.vector.tensor_copy(ns[:, base : base + col], pns[:, :col])
                nc.vector.tensor_scalar_max(
                    ns[:, base : base + col], ns[:, base : base + col], EPS2
                )
                nc.scalar.activation(
                    rw[:, base : base + col], ns[:, base : base + col], AF.Ln
                )
                nc.scalar.activation(
                    rw[:, base : base + col],
                    rw[:, base : base + col],
                    AF.Exp,
                    scale=-0.5,
                )
                for bi in blocks:
                    c0, c1 = cols_of[bi]
                    o0, o1 = BLOCKS[bi]
                    nt = c1 - c0
                    prt = psum.tile([P, 2048], F32, tag="ps")
                    nc.tensor.transpose(
                        prt[:nt, :P], rw[:, base + c0 : base + c1], ident[:]
                    )
                    nc.vector.tensor_copy(
                        rT[:nt, bi * P : bi * P + P], prt[:nt, :P]
                    )
                    # replicate scale rows across all 128 partitions via a
                    # DRAM bounce (HWDGE both ways, dtypes match)
                    nc.sync.dma_start(
                        rt_d[bi, :nt, :], rT[:nt, bi * P : bi * P + P]
                    )
                    src = rt_d[bi, :nt, :].unsqueeze(0).broadcast_to((P, nt, P))
                    dst = bsc[:, o0:o1].rearrange("p (t c) -> p t c", c=P)
                    nc.sync.dma_start(dst, src)
                    # wts = wt * bsc
                    nc.vector.tensor_mul(
                        wts[:, o0:o1], wt[:, o0:o1], bsc[:, o0:o1]
                    )

            w_prep_group(0)
            w_prep_group(1)

            # ---- main loop ------------------------------------------------
            def fill_and_consume(ch, bi, sidx):
                o0, o1 = BLOCKS[bi]
                gw = o1 - o0
                pt = psum.tile([P, 2048], F32, tag="ps")
                lhs = fT[:, ch * P : (ch + 1) * P]
                for b in range(0, gw, MMB):
                    bw = min(MMB, gw - b)
                    nc.tensor.matmul(
                        pt[:, b : b + bw],
                        lhs,
                        wts[:, o0 + b : o0 + b + bw],
                        start=True,
                        stop=True,
                    )
                acc = outt[:, 4 * ch + bi : 4 * ch + bi + 1]
                if bi in ACT_BLOCKS:
                    nc.scalar.activation(
                        pt[:, :gw],
                        pt[:, :gw],
                        AF.Exp,
                        bias=ebias[:],
                        scale=S_SCALE,
                        accum_out=acc,
                    )
                else:
                    ei = eint[sidx][:, :gw]
                    ed = edum[sidx][:, :gw]
                    nc.vector.tensor_scalar(
                        ei, pt[:, :gw], K1S, K2P, OP.mult, OP.add
                    )
                    ev = ei.bitcast(BF16)
                    nc.vector.tensor_scalar(
                        ed, ev, 1.0, 0.0, OP.mult, OP.add, accum_out=acc
                    )

            # pass 1: blocks 0 (ACT) and 2 (DVE) for all chunks
            for ch in range(NCH):
                fill_and_consume(ch, 0, 0)
                fill_and_consume(ch, 2, ch % 2)

            # ---- wl-path (own 128 samples only) --------------------------
            pq5 = pq[:].rearrange("p (k d) -> p k d", d=D)
            nc.vector.tensor_mul(pq5[:, 0, :], fmy[:], fmy[:])
            nc.vector.tensor_mul(pq5[:, 1, :], wl[:], wl[:])
            nc.vector.reduce_sum(nrm2[:], pq5[:, 0:2, :], axis=AX.X)
            nc.vector.tensor_scalar_max(nrm2[:], nrm2[:], EPS2)
            nc.scalar.activation(nrml[:], nrm2[:], AF.Ln)
            nc.scalar.activation(rfw[:], nrml[:], AF.Exp, scale=-0.5)
            nc.vector.tensor_mul(pq5[:, 2, :], fmy[:], wl[:])
            nc.vector.reduce_sum(dots[:], pq5[:, 2:3, :], axis=AX.X)
            nc.vector.tensor_sub(pq5[:, 3, :], fmy[:], wl[:])
            nc.vector.tensor_mul(pq5[:, 4, :], pq5[:, 3, :], pq5[:, 3, :])
            nc.vector.reduce_sum(
                outt[:, 4 * NCH + 1 : 4 * NCH + 2], pq5[:, 4:5, :], axis=AX.X
            )
            nc.vector.tensor_mul(tt[:], dots[:], rfw[:, 0:1])
            nc.vector.tensor_mul(
                outt[:, 4 * NCH : 4 * NCH + 1], tt[:], rfw[:, 1:2]
            )

            # pass 2: blocks 1 (ACT) and 3 (DVE) for all chunks
            for ch in range(NCH):
                fill_and_consume(ch, 1, 0)
                fill_and_consume(ch, 3, ch % 2)

            nc.sync.dma_start(o_d[:], outt[:])

            loop_cm.__exit__(None, None, None)

    _split_multi_waits(nc)
    return nc


_NC_CACHE = None


def _get_program():
    global _NC_CACHE
    if _NC_CACHE is None:
        _NC_CACHE = _build_program()
    return _NC_CACHE


def _build_program_loop(iters):
    return _build_program(loop_iters=iters)


def _host_prepare(feature, weight, label):
    feature = np.ascontiguousarray(np.asarray(feature, dtype=np.float32))
    weight = np.asarray(weight, dtype=np.float32)
    label = np.asarray(label).astype(np.int64)
    wl = np.ascontiguousarray(weight[label])
    w_pad = np.zeros((CPAD, D), dtype=np.float32)
    w_pad[:C] = weight
    in_maps = []
    for k in range(N_CORES):
        shard = w_pad[k * CLOC : (k + 1) * CLOC]
        wt = np.ascontiguousarray(shard.T.astype(ml_dtypes.bfloat16))
        in_maps.append(
            {
                "wt": wt,
                "f": feature,
                "fmy": np.ascontiguousarray(feature[k * P : (k + 1) * P]),
                "wl": np.ascontiguousarray(wl[k * P : (k + 1) * P]),
            }
        )
    return in_maps


def _host_combine(results):
    # device layout: out[p, col]; sample n = ch*128 + p
    outs = [np.asarray(r["o"], dtype=np.float64) for r in results]
    A = sum(o[:, : 4 * NCH].reshape(P, NCH, 4).sum(axis=2) for o in outs)
    A_n = A.T.reshape(N)  # [1024]
    t_n = np.concatenate([o[:, 4 * NCH] for o in outs])  # [1024]
    q_n = np.concatenate([o[:, 4 * NCH + 1] for o in outs])

    # padded classes live in the DVE (Schraudolph) blocks of the last core:
    # each contributes the exact bit-trick value of exp(s*0 - 30).
    pad_term = _schraudolph_exp_np(np.zeros(1))[0]
    S_raw = A_n - NPAD * pad_term
    S_fix = (
        S_raw
        - np.exp(S_SCALE * t_n + EXP_BIAS)
        + np.exp(S_SCALE * (t_n - M_MARGIN) + EXP_BIAS)
    )
    lse = np.log(S_fix) - EXP_BIAS
    target_logit = S_SCALE * (t_n - M_MARGIN)
    loss_lmc = np.mean(lse - target_logit)
    loss_c = 0.5 * np.sum(q_n)
    return np.float32(loss_lmc + LAMBDA * loss_c)


def kernel(feature, weight, label):
    nc = _get_program()
    in_maps = _host_prepare(feature, weight, label)
    res = run_bass_kernel_spmd(nc, in_maps, list(range(N_CORES)))
    return _host_combine(res.results)


def run_sim(feature, weight, label, core=7, trace=False):
    """Simulate a single core and return its raw output tile + in_maps."""
    from concourse.bass_interp import MultiCoreSim

    nc = _get_program()
    in_maps = _host_prepare(feature, weight, label)
    sim = MultiCoreSim(nc, 1, trace=trace)
    for name, arr in in_maps[core].items():
        sim.cores[0].tensor(name)[:] = arr
    sim.simulate()
    return np.array(sim.cores[0].tensor("o")), in_maps
